# revision 2
# baseline (speedup 1.0000x reference)
"""FFM CrossLayer pairwise-interaction kernel for 8x Trainium2 NeuronCores.

Math: out[b] = sum_{i<j} <K[i,f_j,:], K[j,f_i,:]> * x[b,i] * x[b,j]
With W[i,j] = sum_o K[i,f_j,o]*K[j,f_i,o] (symmetric), this equals
    out[b] = 0.5 * x_b^T (W - diag(W)) x_b.

Strategy (8 cores, zero collectives):
  - W is ROW-sharded: core c owns rows Rc = [64c, 64c+64). It builds
    W[Rc,:]^T = wp[j, i_loc] = sum_o t1[j,(i,o)] * bko[j,(i,o)] where
      t1[j,(i,o)]  = K[i, f_j, o]   via one-hot matmul on PE (exact gather)
      bko[j,(i,o)] = K[j, f_i, o]   host-pregathered bf16 input, with the
                                    diagonal entries pre-zeroed (folds the
                                    -diag(W) correction in for free).
  - Phase B runs the FULL batch on every core: YT[i,b] = sum_j W[i,j] xT[j,b]
    (4 stationary W^T tiles x 8 batch chunks), then the partial scalar
    s_c[b] = sum_{i in Rc} YT[i,b] * x[b,i] via a ones-vector matmul.
  - Host gathers: out = 0.5 * sum_c s_c.  No AllReduce, no gpsimd gather.
  - bf16 everywhere off the critical accumulators (PSUM + reduces stay f32).
"""

import sys

import numpy as np

try:  # the grading env may or may not have concourse on sys.path already
    import concourse.bass as bass  # noqa: F401
except ImportError:
    sys.path.insert(0, "/opt/trn_rl_repo")

import ml_dtypes

import concourse.bacc as bacc
import concourse.bass as bass
import concourse.mybir as mybir
import concourse.tile as tile
from concourse.bass_utils import run_bass_kernel_spmd

B, D, F, O = 4096, 512, 64, 64
NC = 8            # cores
R = D // NC       # W rows per core (64)
P = 128           # partitions
NJT = D // P      # j tiles (4)
NBC = B // 512    # batch chunks (8)
CH = 1024         # (i,o) chunk width for phase A
NCH = R * O // CH  # chunks per j-tile (4)
F32 = mybir.dt.float32
BF16 = mybir.dt.bfloat16

_CACHE = {}


def _build_program():
    nc = bacc.Bacc("TRN2", target_bir_lowering=False, debug=False, num_devices=NC)

    eh = nc.dram_tensor("eh", [F, D], BF16, kind="ExternalInput").ap()
    ktR = nc.dram_tensor("ktR", [F, R * O], BF16, kind="ExternalInput").ap()
    bko = nc.dram_tensor("bko", [D, R * O], BF16, kind="ExternalInput").ap()
    xT = nc.dram_tensor("xT", [D, B], BF16, kind="ExternalInput").ap()
    xTR = nc.dram_tensor("xTR", [R, B], BF16, kind="ExternalInput").ap()
    ones = nc.dram_tensor("ones", [R, 1], BF16, kind="ExternalInput").ap()
    outv = nc.dram_tensor("outv", [B], F32, kind="ExternalOutput").ap()

    with tile.TileContext(nc) as tc:
        with (
            tc.tile_pool(name="cst", bufs=1) as cst,
            tc.tile_pool(name="sb", bufs=3) as sb,
            tc.tile_pool(name="wpool", bufs=1) as wpool,
            tc.tile_pool(name="psA", bufs=2, space="PSUM") as psA,
            tc.tile_pool(name="psY", bufs=2, space="PSUM") as psY,
            tc.tile_pool(name="psS", bufs=2, space="PSUM") as psS,
        ):
            # ---- constant loads (order = DMA priority) ----
            eh_sb = cst.tile([F, D], BF16, tag="eh")
            nc.sync.dma_start(eh_sb[:], eh[:])
            ktR_sb = cst.tile([F, R * O], BF16, tag="ktR")
            nc.sync.dma_start(ktR_sb[:], ktR[:])
            bko_sb = []
            for jt in range(NJT):
                t = cst.tile([P, R * O], BF16, tag=f"bko{jt}")
                nc.sync.dma_start(t[:], bko[jt * P : (jt + 1) * P, :])
                bko_sb.append(t)
            xT_sb = []
            for jt in range(NJT):
                t = cst.tile([P, B], BF16, tag=f"xT{jt}")
                nc.sync.dma_start(t[:], xT[jt * P : (jt + 1) * P, :])
                xT_sb.append(t)
            xTR_sb = cst.tile([R, B], BF16, tag="xTR")
            nc.sync.dma_start(xTR_sb[:], xTR[:])
            ones_sb = cst.tile([R, 1], BF16, tag="ones")
            nc.sync.dma_start(ones_sb[:], ones[:])

            # ---- phase A: wp[j, i_loc] = sum_o t1 * bko ----
            wt_sb = []  # W^T tiles, bf16, lhsT for phase B
            for jt in range(NJT):
                wp = wpool.tile([P, R], F32, tag=f"wp{jt}")
                wt = wpool.tile([P, R], BF16, tag=f"wt{jt}")
                wt_sb.append(wt)
                for ch in range(NCH):
                    pt = psA.tile([P, CH], F32, tag="pt")  # 2 PSUM banks
                    for n in range(CH // 512):
                        nc.tensor.matmul(
                            pt[:, n * 512 : (n + 1) * 512],
                            eh_sb[:, jt * P : (jt + 1) * P],
                            ktR_sb[:, ch * CH + n * 512 : ch * CH + (n + 1) * 512],
                            start=True, stop=True,
                        )
                    zt = sb.tile([P, CH], BF16, tag="zt")
                    nc.scalar.copy(zt[:], pt[:])  # ACT: PSUM f32 -> SBUF bf16
                    z2 = sb.tile([P, CH], BF16, tag="z2")
                    nc.vector.tensor_mul(
                        z2[:], zt[:], bko_sb[jt][:, ch * CH : (ch + 1) * CH]
                    )
                    nc.vector.tensor_reduce(
                        wp[:, ch * (CH // O) : (ch + 1) * (CH // O)],
                        z2[:].rearrange("p (i o) -> p i o", o=O),
                        axis=mybir.AxisListType.X, op=mybir.AluOpType.add,
                    )
                nc.vector.tensor_copy(wt[:], wp[:])  # f32 -> bf16, tiny

            # ---- phase B: YT = W^T.T @ xT, s = ones.T @ (YT * xTR) ----
            s_sb = wpool.tile([1, B], F32, tag="s")
            for bc in range(NBC):
                yp = psY.tile([R, 512], F32, tag="yp")
                for jt in range(NJT):
                    nc.tensor.matmul(
                        yp[:], wt_sb[jt][:], xT_sb[jt][:, bc * 512 : (bc + 1) * 512],
                        start=(jt == 0), stop=(jt == NJT - 1),
                    )
                pb = sb.tile([R, 512], BF16, tag="pb")
                nc.vector.tensor_mul(
                    pb[:], yp[:], xTR_sb[:, bc * 512 : (bc + 1) * 512]
                )
                sp = psS.tile([1, 512], F32, tag="sp")
                nc.tensor.matmul(sp[:], ones_sb[:], pb[:], start=True, stop=True)
                nc.scalar.copy(s_sb[:, bc * 512 : (bc + 1) * 512], sp[:])
            nc.sync.dma_start(outv[:], s_sb[:])

    nc.compile()
    return nc


def _host_prep(x, kern, field_ids):
    x = np.asarray(x, dtype=np.float32)
    k = np.asarray(kern, dtype=np.float32)
    fid = np.asarray(field_ids).astype(np.int64).ravel()
    assert x.shape == (B, D) and k.shape == (D, F, O) and fid.shape == (D,)

    bf = ml_dtypes.bfloat16
    eh = (fid[None, :] == np.arange(F)[:, None]).astype(bf)       # [F, D]
    xT_b = np.ascontiguousarray(x.T).astype(bf)                    # [D, B]
    ones = np.ones((R, 1), dtype=bf)

    in_maps = []
    for c in range(NC):
        Rc = slice(c * R, (c + 1) * R)
        ktR_c = np.ascontiguousarray(
            k[Rc].transpose(1, 0, 2).reshape(F, R * O)
        ).astype(bf)                                               # [F, (i,o)]
        bko_c = k[:, fid[Rc], :].copy()                            # [D, R, O]
        for il in range(R):
            bko_c[c * R + il, il, :] = 0.0                         # fold -diag(W)
        in_maps.append({
            "eh": eh, "ktR": ktR_c,
            "bko": bko_c.reshape(D, R * O).astype(bf),
            "xT": xT_b,
            "xTR": np.ascontiguousarray(xT_b[Rc]),
            "ones": ones,
        })
    return in_maps


def kernel(x, kernel, field_ids):
    if "nc" not in _CACHE:
        _CACHE["nc"] = _build_program()
    nc = _CACHE["nc"]
    in_maps = _host_prep(x, kernel, field_ids)
    res = run_bass_kernel_spmd(nc, in_maps, core_ids=list(range(NC)))
    out = np.zeros(B, dtype=np.float64)
    for c in range(NC):
        out += np.asarray(res.results[c]["outv"], dtype=np.float64).ravel()
    return (0.5 * out).astype(np.float32)
